# revision 32
# baseline (speedup 1.0000x reference)
"""Trainium2 Bass kernel for nn_AttnHead_81028853006993.

LayerNorm + affine + fused QKV + 4-head attention with gathered relative-position
mask + output projection, for x:[8, 2048, 512] f32.

Sharding: data-parallel over batch — 8 batches onto 8 NeuronCores, no collectives.

v2 design: scores are computed TRANSPOSED (S^T[k, q] via K^T-tile-stationary
matmuls) so that exp(S^T) is directly the stationary operand PV needs — this
eliminates the per-(query-tile, head) PE transposes of the probability matrix
and the PE-based additive-mask injection of v1 (each ~131k PE cycles). The
mask is applied multiplicatively (exp(s) * m01) on the DVE in bf16 (2x mode),
and the softmax denominator falls out of the PV matmul itself via a ones
column appended to V. Matmuls in bf16; stats/softmax accumulation in f32.
"""

import os
import sys

import numpy as np

for _p in ("/opt/trn_rl_repo",):
    if _p not in sys.path:
        sys.path.insert(0, _p)

import ml_dtypes  # noqa: E402

B, T, N = 8, 2048, 512
H, HD = 4, 128
P = 128
NT = T // P  # 16 token tiles
KC = N // P  # 4 embed chunks
FQK = 2 * N // P  # 8 feature chunks for fused QK
HDP = 132  # padded per-head V row: 128 V cols + 1 ones col + pad
EPS = 1e-5

LAST_RESULTS = None
_CACHE = {}


def _build_nc():
    import concourse.bacc as bacc
    import concourse.mybir as mybir
    import concourse.tile as tile
    from concourse.bass import AP, ts
    from concourse.masks import make_identity

    f32 = mybir.dt.float32
    bf16 = mybir.dt.bfloat16
    FI = mybir.ActivationFunctionType

    nc = bacc.Bacc("TRN2", target_bir_lowering=False, debug=False, num_devices=8)

    x_d = nc.dram_tensor("x", [T, N], f32, kind="ExternalInput")
    # multiplicative mask, transposed + pre-tiled on host:
    # maskt[qq, p, m, q'] = mask01[qq*512+q', m*128+p] — one contiguous
    # 16KB run per partition per qq block (128 DMA descriptors instead of 2048)
    mask_d = nc.dram_tensor("maskt", [4, P, NT, 512], bf16, kind="ExternalInput")
    wqk_d = nc.dram_tensor("wqk", [N, 2 * N], bf16, kind="ExternalInput")
    wv_d = nc.dram_tensor("wv", [N, N], bf16, kind="ExternalInput")
    wp_d = nc.dram_tensor("wproj", [N, N], bf16, kind="ExternalInput")
    bqk_d = nc.dram_tensor("bqk", [2 * N], f32, kind="ExternalInput")
    # obias pre-broadcast on host to [P, N] — a partition-stride-0 broadcast
    # DMA (or a 1-partition DMA) costs >10us of descriptor-generation time
    ob_d = nc.dram_tensor("obias", [P, N], f32, kind="ExternalInput")
    out_d = nc.dram_tensor("out", [T, N], f32, kind="ExternalOutput")

    with tile.TileContext(nc) as tc:
        from contextlib import ExitStack

        with ExitStack() as ctx:
            singles = ctx.enter_context(tc.tile_pool(name="singles", bufs=1))
            big = ctx.enter_context(tc.tile_pool(name="big", bufs=1))
            xtp = ctx.enter_context(tc.tile_pool(name="xtp", bufs=2))
            lnx = ctx.enter_context(tc.tile_pool(name="lnx", bufs=3))
            smallp = ctx.enter_context(tc.tile_pool(name="smallp", bufs=8))
            maskp = ctx.enter_context(tc.tile_pool(name="maskp", bufs=2))
            probsp = ctx.enter_context(tc.tile_pool(name="probsp", bufs=2))
            expp = ctx.enter_context(tc.tile_pool(name="expp", bufs=4))
            attnp = ctx.enter_context(tc.tile_pool(name="attnp", bufs=8))
            attntp = ctx.enter_context(tc.tile_pool(name="attntp", bufs=2))
            outp = ctx.enter_context(tc.tile_pool(name="outp", bufs=3))
            # PSUM: ps_sc 2x[128,1024]f32 (4 banks) + ps_tp 2x[128,512]bf16
            # (2 banks) + ps_v 2x[128,512]f32 (2 banks) = 8 banks
            ps_sc = ctx.enter_context(tc.tile_pool(name="ps_sc", bufs=2, space="PSUM"))
            ps_tp = ctx.enter_context(tc.tile_pool(name="ps_tp", bufs=2, space="PSUM"))
            ps_v = ctx.enter_context(tc.tile_pool(name="ps_v", bufs=2, space="PSUM"))

            # ---- identity first (gpsimd); first x tiles via the fast SP
            # HWDGE path ahead of the weights, rest via gpsimd, so LayerNorm
            # starts as early as possible ----
            ident_b = singles.tile([P, P], bf16)
            make_identity(nc, ident_b)
            wqk_sb = singles.tile([P, KC, 2 * N], bf16)
            nc.sync.dma_start(
                out=wqk_sb, in_=wqk_d.ap().rearrange("(kc p) f -> p kc f", p=P)
            )
            x_tiles = []
            for i in range(NT):
                x_tile = lnx.tile(
                    [P, N], f32, tag="x_tile", bufs=NT, name=f"x_tile_{i}"
                )
                eng = nc.sync if i < 3 else nc.gpsimd
                eng.dma_start(out=x_tile, in_=x_d.ap()[ts(i, P), :])
                x_tiles.append(x_tile)

            # ---- constants / weights ----
            eps_t = singles.tile([P, 1], f32)
            nc.vector.memset(eps_t, EPS)

            wv_sb = singles.tile([P, KC, N], bf16)
            nc.sync.dma_start(
                out=wv_sb, in_=wv_d.ap().rearrange("(kc p) f -> p kc f", p=P)
            )
            wp_sb = singles.tile([P, KC, N], bf16)
            nc.sync.dma_start(
                out=wp_sb, in_=wp_d.ap().rearrange("(kc p) f -> p kc f", p=P)
            )
            bqk_sb = singles.tile([P, FQK], f32)
            nc.sync.dma_start(
                out=bqk_sb, in_=bqk_d.ap().rearrange("(fc p) -> p fc", p=P)
            )
            ob_bc = singles.tile([P, N], f32)
            nc.sync.dma_start(out=ob_bc, in_=ob_d.ap())

            qkT = big.tile([P, FQK, T], bf16)  # Q^T,K^T feature-major
            # V token-major, per head 128 cols + ones col (for softmax denom)
            vaug = big.tile([P, NT, H, HDP], bf16)
            nc.vector.memset(vaug[:, :, :, HD : HD + 1], 1.0)

            def alt_copy(idx, out, in_):
                if idx % 2 == 0:
                    nc.vector.tensor_copy(out=out, in_=in_)
                else:
                    nc.scalar.copy(out=out, in_=in_)

            # ---- fused LN + QKV phase, per 512-token chunk ----
            copy_flip = 0
            for tj in range(4):
                xtc = xtp.tile([P, KC, 4 * P], bf16)  # x-hat^T for this token chunk
                for s in range(4):
                    i = tj * 4 + s
                    x_tile = x_tiles[i]
                    stats = smallp.tile([P, 6], f32)
                    nc.vector.bn_stats(out=stats, in_=x_tile)
                    mv = smallp.tile([P, 2], f32)
                    nc.vector.bn_aggr(out=mv, in_=stats)
                    sig = smallp.tile([P, 1], f32)
                    nc.scalar.activation(
                        out=sig, in_=mv[:, 1:2], func=FI.Sqrt, bias=eps_t
                    )
                    rstd = smallp.tile([P, 1], f32)
                    nc.vector.reciprocal(out=rstd, in_=sig)
                    # x-hat = (x - mean) * rstd, cast to bf16
                    xh = lnx.tile([P, N], bf16)
                    nc.vector.tensor_scalar(
                        out=xh,
                        in0=x_tile,
                        scalar1=mv[:, 0:1],
                        scalar2=rstd,
                        op0=mybir.AluOpType.subtract,
                        op1=mybir.AluOpType.mult,
                    )
                    ps_x = ps_tp.tile([P, 4 * P], bf16, tag="pst")
                    for kc in range(KC):
                        nc.tensor.matmul(
                            ps_x[:, ts(kc, P)],
                            xh[:, ts(kc, P)],
                            ident_b,
                            start=(kc == 0),
                            stop=(kc == KC - 1),
                            is_transpose=True,
                        )
                    alt_copy(
                        copy_flip,
                        xtc[:, :, ts(s, P)],
                        ps_x.rearrange("p (kc q) -> p kc q", kc=KC),
                    )
                    copy_flip += 1
                # QK^T for this token chunk: out[feat, tok]
                for g in range(4):
                    ps = ps_sc.tile([P, 1024], f32, tag="psc")
                    for half in range(2):
                        fc = g * 2 + half
                        for kc in range(KC):
                            nc.tensor.matmul(
                                ps[:, ts(half, 512)],
                                wqk_sb[:, kc, ts(fc, P)],
                                xtc[:, kc, :],
                                start=(kc == 0),
                                stop=(kc == KC - 1),
                            )
                    for half in range(2):
                        fc = g * 2 + half
                        if copy_flip % 2 == 0:
                            nc.vector.tensor_scalar_add(
                                out=qkT[:, fc, ts(tj, 512)],
                                in0=ps[:, ts(half, 512)],
                                scalar1=bqk_sb[:, fc : fc + 1],
                            )
                        else:
                            nc.scalar.activation(
                                out=qkT[:, fc, ts(tj, 512)],
                                in_=ps[:, ts(half, 512)],
                                func=FI.Identity,
                                bias=bqk_sb[:, fc : fc + 1],
                            )
                        copy_flip += 1
                # V for this token chunk: out[tok, feat] -> vaug (bf16)
                for s in range(4):
                    tm = tj * 4 + s
                    ps2 = ps_v.tile([P, N], f32, tag="psv")
                    for kc in range(KC):
                        nc.tensor.matmul(
                            ps2,
                            xtc[:, kc, ts(s, P)],
                            wv_sb[:, kc, :],
                            start=(kc == 0),
                            stop=(kc == KC - 1),
                        )
                    alt_copy(
                        copy_flip,
                        vaug[:, tm, :, 0:HD],
                        ps2.rearrange("p (h d) -> p h d", h=H),
                    )
                    copy_flip += 1

            # ---- attention phase (transposed scores), per 512-query block ----
            def emit_pv_chunk(pend, mm):
                """8 PV matmuls for the previous head's probs: j = mm//2,
                m in [8*(mm%2), 8*(mm%2)+8). Group per j accumulates over all
                16 k-tiles into psum [q128, HD+1]; col HD is the denominator."""
                j = mm // 2
                half = mm % 2
                if half == 0:
                    pend["pv"][j] = ps_v.tile(
                        [P, N], f32, tag="psv", name=f"pv_{pend['qq']}_{pend['h']}_{j}"
                    )
                pv = pend["pv"][j]
                pT = pend["probsT"]
                hh = pend["h"]
                for mi in range(8):
                    m = half * 8 + mi
                    nc.tensor.matmul(
                        pv[:, 0 : HD + 1],
                        pT[:, m, ts(j, P)],
                        vaug[:, m, hh, 0 : HD + 1],
                        start=(m == 0),
                        stop=(m == 15),
                    )
                if half == 1:
                    recip = smallp.tile(
                        [P, 1], f32, tag="rc", name=f"rc_{pend['qq']}_{hh}_{j}"
                    )
                    nc.vector.reciprocal(out=recip, in_=pv[:, HD : HD + 1])
                    nc.vector.tensor_scalar_mul(
                        out=pend["attn"][j][:, ts(hh, HD)],
                        in0=pv[:, 0:HD],
                        scalar1=recip,
                    )

            def emit_proj_j(qq, attn_tiles, j):
                i = qq * 4 + j
                ps_at = ps_tp.tile([P, N], bf16, tag="pst", name=f"psat_{i}")
                for k in range(KC):
                    nc.tensor.matmul(
                        ps_at[:, ts(k, P)],
                        attn_tiles[j][:, ts(k, P)],
                        ident_b,
                        start=(k == 0),
                        stop=(k == KC - 1),
                        is_transpose=True,
                    )
                attnT = attntp.tile([P, KC, P], bf16, tag="attnT", name=f"attnT_{i}")
                nc.vector.tensor_copy(
                    out=attnT, in_=ps_at.rearrange("p (kc q) -> p kc q", kc=KC)
                )
                ps_pr = ps_v.tile([P, N], f32, tag="psv", name=f"pspr_{i}")
                for c in range(KC):
                    nc.tensor.matmul(
                        ps_pr,
                        attnT[:, c, :],
                        wp_sb[:, c, :],
                        start=(c == 0),
                        stop=(c == KC - 1),
                    )
                out_sb = outp.tile([P, N], f32, tag="out_sb", name=f"out_sb_{i}")
                nc.vector.tensor_tensor(
                    out=out_sb, in0=ps_pr, in1=ob_bc, op=mybir.AluOpType.add
                )
                nc.sync.dma_start(out=out_d.ap()[ts(i, P), :], in_=out_sb)

            pend = None
            pending_proj = None  # (qq, attn_tiles) with PV complete, proj due
            for qq in range(4):
                maskT_sb = maskp.tile([P, NT, 512], bf16, tag="maskT", name=f"maskT_{qq}")
                nc.sync.dma_start(out=maskT_sb, in_=mask_d.ap()[qq])
                attn_tiles = [
                    attnp.tile([P, N], bf16, tag="attn", name=f"attn_{qq}_{j}") for j in range(4)
                ]
                for h in range(H):
                    probsT = probsp.tile([P, NT, 512], bf16, tag="pT", name=f"pT_{qq}_{h}")
                    for mm in range(8):
                        ps_s = ps_sc.tile(
                            [P, 1024], f32, tag="psc", name=f"ps_s_{qq}_{h}_{mm}"
                        )
                        for u in range(2):
                            m = 2 * mm + u
                            nc.tensor.matmul(
                                ps_s[:, ts(u, 512)],
                                qkT[:, H + h, ts(m, P)],
                                qkT[:, h, ts(qq, 512)],
                                start=True,
                                stop=True,
                            )
                        et = expp.tile([P, 1024], bf16, tag="et", name=f"et_{qq}_{h}_{mm}")
                        nc.scalar.activation(out=et, in_=ps_s, func=FI.Exp)
                        nc.vector.tensor_tensor(
                            out=probsT[:, 2 * mm : 2 * mm + 2, :],
                            in0=et.rearrange("p (m q) -> p m q", m=2),
                            in1=maskT_sb[:, 2 * mm : 2 * mm + 2, :],
                            op=mybir.AluOpType.mult,
                        )
                        if pend is not None:
                            emit_pv_chunk(pend, mm)
                        # spread the previous qq's projection one j-block per
                        # head (at mm==5, after that j's PV is finalized) so
                        # per-head PE work stays balanced against the exps
                        if pending_proj is not None and mm == 5:
                            pq, pattn, nj = pending_proj
                            emit_proj_j(pq, pattn, nj)
                            pending_proj = None if nj == 3 else (pq, pattn, nj + 1)
                    pend = {
                        "probsT": probsT,
                        "h": h,
                        "attn": attn_tiles,
                        "qq": qq,
                        "pv": {},
                    }
                    if h == H - 1 and qq < 3:
                        pending_proj = (qq, attn_tiles, 0)
            # tail: flush PV for (qq=3, h=3) and its projection
            for mm in range(8):
                emit_pv_chunk(pend, mm)
            for j in range(4):
                emit_proj_j(3, pend["attn"], j)

    nc.compile()
    return nc


def _get_nc():
    if "nc" not in _CACHE:
        _CACHE["nc"] = _build_nc()
    return _CACHE["nc"]


def _prep_host(x, pos_emb, ln_w, ln_b, aff_w, aff_b, W_qkv, mask_table, W_proj):
    f = np.float32
    bf = ml_dtypes.bfloat16
    x = np.asarray(x, f)
    pos_emb = np.asarray(pos_emb)
    ln_w = np.asarray(ln_w, f)
    ln_b = np.asarray(ln_b, f)
    aff_w = np.asarray(aff_w, f)
    aff_b = np.asarray(aff_b, f)
    W_qkv = np.asarray(W_qkv, f)
    mask_table = np.asarray(mask_table)
    W_proj = np.asarray(W_proj, f)

    s = ln_w * aff_w
    c = ln_b * aff_w + aff_b
    Wf = (s[:, None] * W_qkv).astype(f)
    bf_ = (c @ W_qkv).astype(f)
    scale = f(1.0 / np.sqrt(HD))
    Wqk = np.concatenate([Wf[:, :N] * scale, Wf[:, N : 2 * N]], axis=1)
    Wqk = np.ascontiguousarray(Wqk).astype(bf)
    bqk = np.concatenate([bf_[:N] * scale, bf_[N : 2 * N]]).astype(f)
    Wv = np.ascontiguousarray(Wf[:, 2 * N :]).astype(bf)
    bv = bf_[2 * N :]
    obias = np.ascontiguousarray(
        np.broadcast_to((bv @ W_proj).astype(f), (P, N))
    )
    Wp = np.ascontiguousarray(W_proj).astype(bf)
    # transposed multiplicative mask, pre-tiled:
    # maskt[qq, p, m, q'] = mask01[qq*512+q', m*128+p]
    maskmt = np.where(mask_table[pos_emb], f(1.0), f(0.0)).T  # [k, q]
    maskt = np.ascontiguousarray(
        maskmt.reshape(NT, P, 4, 512).transpose(2, 1, 0, 3)
    ).astype(bf)
    return x, maskt, Wqk, Wv, Wp, bqk, obias


def _install_ntff_hook():
    """Provide the antenv.axon_hooks shim missing from this image so
    run_bass_kernel_spmd(trace=True) can capture NTFF profiles."""
    import types

    try:
        from antenv.axon_hooks import get_axon_ntff_profile_hook  # noqa: F401

        return
    except ImportError:
        pass
    try:
        import antenv
        from trn_agent_boot.trn_boot import _ntff_profile_via_ctypes

        hook = _ntff_profile_via_ctypes("/opt/axon/libaxon_pjrt.so")
        mod = types.ModuleType("antenv.axon_hooks")
        _h = [hook]
        mod.set_axon_ntff_profile_hook = lambda h: _h.__setitem__(0, h)
        mod.get_axon_ntff_profile_hook = lambda: _h[0]
        sys.modules["antenv.axon_hooks"] = mod
        antenv.axon_hooks = mod
    except Exception as e:  # pragma: no cover
        print(f"ntff hook install failed: {e}")


def kernel(x, pos_emb, ln_w, ln_b, aff_w, aff_b, W_qkv, mask_table, W_proj):
    global LAST_RESULTS
    from concourse.bass_utils import run_bass_kernel_spmd

    x, maskt, Wqk, Wv, Wp, bqk, obias = _prep_host(
        x, pos_emb, ln_w, ln_b, aff_w, aff_b, W_qkv, mask_table, W_proj
    )
    nc = _get_nc()
    in_maps = [
        {
            "x": np.ascontiguousarray(x[i]),
            "maskt": maskt,
            "wqk": Wqk,
            "wv": Wv,
            "wproj": Wp,
            "bqk": bqk,
            "obias": obias,
        }
        for i in range(B)
    ]
    trace = bool(int(os.environ.get("ATTN_TRACE", "0")))
    if trace:
        _install_ntff_hook()
    res = run_bass_kernel_spmd(
        nc, in_maps, core_ids=list(range(B)), trace=trace
    )
    LAST_RESULTS = res
    out = np.stack([np.asarray(r["out"]) for r in res.results], axis=0)
    return out.astype(np.float32)


# revision 33
# speedup vs baseline: 1.1862x; 1.1862x over previous
"""Trainium2 Bass kernel for nn_AttnHead_81028853006993.

LayerNorm + affine + fused QKV + 4-head attention with gathered relative-position
mask + output projection, for x:[8, 2048, 512] f32.

Sharding: data-parallel over batch — 8 batches onto 8 NeuronCores, no collectives.

v2 design: scores are computed TRANSPOSED (S^T[k, q] via K^T-tile-stationary
matmuls) so that exp(S^T) is directly the stationary operand PV needs — this
eliminates the per-(query-tile, head) PE transposes of the probability matrix
and the PE-based additive-mask injection of v1 (each ~131k PE cycles). The
mask is applied multiplicatively (exp(s) * m01) on the DVE in bf16 (2x mode),
and the softmax denominator falls out of the PV matmul itself via a ones
column appended to V. Matmuls in bf16; stats/softmax accumulation in f32.
"""

import os
import sys

import numpy as np

for _p in ("/opt/trn_rl_repo",):
    if _p not in sys.path:
        sys.path.insert(0, _p)

import ml_dtypes  # noqa: E402

B, T, N = 8, 2048, 512
H, HD = 4, 128
P = 128
NT = T // P  # 16 token tiles
KC = N // P  # 4 embed chunks
FQK = 2 * N // P  # 8 feature chunks for fused QK
HDP = 132  # padded per-head V row: 128 V cols + 1 ones col + pad
EPS = 1e-5

LAST_RESULTS = None
_CACHE = {}


def _build_nc():
    import concourse.bacc as bacc
    import concourse.mybir as mybir
    import concourse.tile as tile
    from concourse.bass import AP, ts
    from concourse.masks import make_identity

    f32 = mybir.dt.float32
    bf16 = mybir.dt.bfloat16
    FI = mybir.ActivationFunctionType

    nc = bacc.Bacc("TRN2", target_bir_lowering=False, debug=False, num_devices=8)

    x_d = nc.dram_tensor("x", [T, N], f32, kind="ExternalInput")
    # multiplicative mask, transposed + pre-tiled on host:
    # maskt[qq, p, m, q'] = mask01[qq*512+q', m*128+p] — one contiguous
    # 16KB run per partition per qq block (128 DMA descriptors instead of 2048)
    mask_d = nc.dram_tensor("maskt", [4, P, NT, 512], bf16, kind="ExternalInput")
    # wqk pre-tiled on host to [KC, P, 2N]: each partition line is one
    # contiguous 4KB DRAM run, so the load streams at full bandwidth
    wqk_d = nc.dram_tensor("wqk", [KC, P, 2 * N], bf16, kind="ExternalInput")
    wv_d = nc.dram_tensor("wv", [N, N], bf16, kind="ExternalInput")
    wp_d = nc.dram_tensor("wproj", [N, N], bf16, kind="ExternalInput")
    bqk_d = nc.dram_tensor("bqk", [2 * N], f32, kind="ExternalInput")
    # obias pre-broadcast on host to [P, N] — a partition-stride-0 broadcast
    # DMA (or a 1-partition DMA) costs >10us of descriptor-generation time
    ob_d = nc.dram_tensor("obias", [P, N], f32, kind="ExternalInput")
    out_d = nc.dram_tensor("out", [T, N], f32, kind="ExternalOutput")

    with tile.TileContext(nc) as tc:
        from contextlib import ExitStack

        with ExitStack() as ctx:
            singles = ctx.enter_context(tc.tile_pool(name="singles", bufs=1))
            big = ctx.enter_context(tc.tile_pool(name="big", bufs=1))
            xtp = ctx.enter_context(tc.tile_pool(name="xtp", bufs=2))
            lnx = ctx.enter_context(tc.tile_pool(name="lnx", bufs=3))
            smallp = ctx.enter_context(tc.tile_pool(name="smallp", bufs=8))
            maskp = ctx.enter_context(tc.tile_pool(name="maskp", bufs=2))
            probsp = ctx.enter_context(tc.tile_pool(name="probsp", bufs=2))
            expp = ctx.enter_context(tc.tile_pool(name="expp", bufs=4))
            attnp = ctx.enter_context(tc.tile_pool(name="attnp", bufs=8))
            attntp = ctx.enter_context(tc.tile_pool(name="attntp", bufs=2))
            outp = ctx.enter_context(tc.tile_pool(name="outp", bufs=3))
            # PSUM: ps_sc 2x[128,1024]f32 (4 banks) + ps_tp 2x[128,512]bf16
            # (2 banks) + ps_v 2x[128,512]f32 (2 banks) = 8 banks
            ps_sc = ctx.enter_context(tc.tile_pool(name="ps_sc", bufs=2, space="PSUM"))
            ps_tp = ctx.enter_context(tc.tile_pool(name="ps_tp", bufs=2, space="PSUM"))
            ps_v = ctx.enter_context(tc.tile_pool(name="ps_v", bufs=2, space="PSUM"))

            # ---- identity first (gpsimd); first x tiles via the fast SP
            # HWDGE path ahead of the weights, rest via gpsimd, so LayerNorm
            # starts as early as possible ----
            ident_b = singles.tile([P, P], bf16)
            make_identity(nc, ident_b)
            wqk_sb = singles.tile([P, KC, 2 * N], bf16)
            for kc in range(KC):
                nc.sync.dma_start(out=wqk_sb[:, kc, :], in_=wqk_d.ap()[kc])
            x_tiles = []
            for i in range(NT):
                x_tile = lnx.tile(
                    [P, N], f32, tag="x_tile", bufs=NT, name=f"x_tile_{i}"
                )
                eng = nc.sync if i < 3 else nc.gpsimd
                eng.dma_start(out=x_tile, in_=x_d.ap()[ts(i, P), :])
                x_tiles.append(x_tile)

            # ---- constants / weights ----
            eps_t = singles.tile([P, 1], f32)
            nc.vector.memset(eps_t, EPS)

            wv_sb = singles.tile([P, KC, N], bf16)
            nc.sync.dma_start(
                out=wv_sb, in_=wv_d.ap().rearrange("(kc p) f -> p kc f", p=P)
            )
            wp_sb = singles.tile([P, KC, N], bf16)
            nc.sync.dma_start(
                out=wp_sb, in_=wp_d.ap().rearrange("(kc p) f -> p kc f", p=P)
            )
            bqk_sb = singles.tile([P, FQK], f32)
            nc.sync.dma_start(
                out=bqk_sb, in_=bqk_d.ap().rearrange("(fc p) -> p fc", p=P)
            )
            ob_bc = singles.tile([P, N], f32)
            nc.sync.dma_start(out=ob_bc, in_=ob_d.ap())

            qkT = big.tile([P, FQK, T], bf16)  # Q^T,K^T feature-major
            # V token-major, per head 128 cols + ones col (for softmax denom)
            vaug = big.tile([P, NT, H, HDP], bf16)
            nc.vector.memset(vaug[:, :, :, HD : HD + 1], 1.0)

            def alt_copy(idx, out, in_):
                if idx % 2 == 0:
                    nc.vector.tensor_copy(out=out, in_=in_)
                else:
                    nc.scalar.copy(out=out, in_=in_)

            # ---- fused LN + QKV phase, per 512-token chunk ----
            copy_flip = 0
            for tj in range(4):
                xtc = xtp.tile([P, KC, 4 * P], bf16)  # x-hat^T for this token chunk
                for s in range(4):
                    i = tj * 4 + s
                    x_tile = x_tiles[i]
                    stats = smallp.tile([P, 6], f32)
                    nc.vector.bn_stats(out=stats, in_=x_tile)
                    mv = smallp.tile([P, 2], f32)
                    nc.vector.bn_aggr(out=mv, in_=stats)
                    sig = smallp.tile([P, 1], f32)
                    nc.scalar.activation(
                        out=sig, in_=mv[:, 1:2], func=FI.Sqrt, bias=eps_t
                    )
                    rstd = smallp.tile([P, 1], f32)
                    nc.vector.reciprocal(out=rstd, in_=sig)
                    # x-hat = (x - mean) * rstd, cast to bf16
                    xh = lnx.tile([P, N], bf16)
                    nc.vector.tensor_scalar(
                        out=xh,
                        in0=x_tile,
                        scalar1=mv[:, 0:1],
                        scalar2=rstd,
                        op0=mybir.AluOpType.subtract,
                        op1=mybir.AluOpType.mult,
                    )
                    ps_x = ps_tp.tile([P, 4 * P], bf16, tag="pst")
                    for kc in range(KC):
                        nc.tensor.matmul(
                            ps_x[:, ts(kc, P)],
                            xh[:, ts(kc, P)],
                            ident_b,
                            start=(kc == 0),
                            stop=(kc == KC - 1),
                            is_transpose=True,
                        )
                    alt_copy(
                        copy_flip,
                        xtc[:, :, ts(s, P)],
                        ps_x.rearrange("p (kc q) -> p kc q", kc=KC),
                    )
                    copy_flip += 1
                # QK^T for this token chunk: out[feat, tok]
                for g in range(4):
                    ps = ps_sc.tile([P, 1024], f32, tag="psc")
                    for half in range(2):
                        fc = g * 2 + half
                        for kc in range(KC):
                            nc.tensor.matmul(
                                ps[:, ts(half, 512)],
                                wqk_sb[:, kc, ts(fc, P)],
                                xtc[:, kc, :],
                                start=(kc == 0),
                                stop=(kc == KC - 1),
                            )
                    for half in range(2):
                        fc = g * 2 + half
                        if copy_flip % 2 == 0:
                            nc.vector.tensor_scalar_add(
                                out=qkT[:, fc, ts(tj, 512)],
                                in0=ps[:, ts(half, 512)],
                                scalar1=bqk_sb[:, fc : fc + 1],
                            )
                        else:
                            nc.scalar.activation(
                                out=qkT[:, fc, ts(tj, 512)],
                                in_=ps[:, ts(half, 512)],
                                func=FI.Identity,
                                bias=bqk_sb[:, fc : fc + 1],
                            )
                        copy_flip += 1
                # V for this token chunk: out[tok, feat] -> vaug (bf16)
                for s in range(4):
                    tm = tj * 4 + s
                    ps2 = ps_v.tile([P, N], f32, tag="psv")
                    for kc in range(KC):
                        nc.tensor.matmul(
                            ps2,
                            xtc[:, kc, ts(s, P)],
                            wv_sb[:, kc, :],
                            start=(kc == 0),
                            stop=(kc == KC - 1),
                        )
                    alt_copy(
                        copy_flip,
                        vaug[:, tm, :, 0:HD],
                        ps2.rearrange("p (h d) -> p h d", h=H),
                    )
                    copy_flip += 1

            # ---- attention phase (transposed scores), per 512-query block ----
            def emit_pv_chunk(pend, mm):
                """8 PV matmuls for the previous head's probs: j = mm//2,
                m in [8*(mm%2), 8*(mm%2)+8). Group per j accumulates over all
                16 k-tiles into psum [q128, HD+1]; col HD is the denominator."""
                j = mm // 2
                half = mm % 2
                if half == 0:
                    pend["pv"][j] = ps_v.tile(
                        [P, N], f32, tag="psv", name=f"pv_{pend['qq']}_{pend['h']}_{j}"
                    )
                pv = pend["pv"][j]
                pT = pend["probsT"]
                hh = pend["h"]
                for mi in range(8):
                    m = half * 8 + mi
                    nc.tensor.matmul(
                        pv[:, 0 : HD + 1],
                        pT[:, m, ts(j, P)],
                        vaug[:, m, hh, 0 : HD + 1],
                        start=(m == 0),
                        stop=(m == 15),
                    )
                if half == 1:
                    recip = smallp.tile(
                        [P, 1], f32, tag="rc", name=f"rc_{pend['qq']}_{hh}_{j}"
                    )
                    nc.vector.reciprocal(out=recip, in_=pv[:, HD : HD + 1])
                    nc.vector.tensor_scalar_mul(
                        out=pend["attn"][j][:, ts(hh, HD)],
                        in0=pv[:, 0:HD],
                        scalar1=recip,
                    )

            def emit_proj_j(qq, attn_tiles, j):
                i = qq * 4 + j
                ps_at = ps_tp.tile([P, N], bf16, tag="pst", name=f"psat_{i}")
                for k in range(KC):
                    nc.tensor.matmul(
                        ps_at[:, ts(k, P)],
                        attn_tiles[j][:, ts(k, P)],
                        ident_b,
                        start=(k == 0),
                        stop=(k == KC - 1),
                        is_transpose=True,
                    )
                attnT = attntp.tile([P, KC, P], bf16, tag="attnT", name=f"attnT_{i}")
                nc.vector.tensor_copy(
                    out=attnT, in_=ps_at.rearrange("p (kc q) -> p kc q", kc=KC)
                )
                ps_pr = ps_v.tile([P, N], f32, tag="psv", name=f"pspr_{i}")
                for c in range(KC):
                    nc.tensor.matmul(
                        ps_pr,
                        attnT[:, c, :],
                        wp_sb[:, c, :],
                        start=(c == 0),
                        stop=(c == KC - 1),
                    )
                out_sb = outp.tile([P, N], f32, tag="out_sb", name=f"out_sb_{i}")
                nc.vector.tensor_tensor(
                    out=out_sb, in0=ps_pr, in1=ob_bc, op=mybir.AluOpType.add
                )
                nc.sync.dma_start(out=out_d.ap()[ts(i, P), :], in_=out_sb)

            pend = None
            pending_proj = None  # (qq, attn_tiles) with PV complete, proj due
            for qq in range(4):
                maskT_sb = maskp.tile([P, NT, 512], bf16, tag="maskT", name=f"maskT_{qq}")
                nc.sync.dma_start(out=maskT_sb, in_=mask_d.ap()[qq])
                attn_tiles = [
                    attnp.tile([P, N], bf16, tag="attn", name=f"attn_{qq}_{j}") for j in range(4)
                ]
                for h in range(H):
                    probsT = probsp.tile([P, NT, 512], bf16, tag="pT", name=f"pT_{qq}_{h}")
                    for mm in range(8):
                        ps_s = ps_sc.tile(
                            [P, 1024], f32, tag="psc", name=f"ps_s_{qq}_{h}_{mm}"
                        )
                        for u in range(2):
                            m = 2 * mm + u
                            nc.tensor.matmul(
                                ps_s[:, ts(u, 512)],
                                qkT[:, H + h, ts(m, P)],
                                qkT[:, h, ts(qq, 512)],
                                start=True,
                                stop=True,
                            )
                        et = expp.tile([P, 1024], bf16, tag="et", name=f"et_{qq}_{h}_{mm}")
                        nc.scalar.activation(out=et, in_=ps_s, func=FI.Exp)
                        nc.vector.tensor_tensor(
                            out=probsT[:, 2 * mm : 2 * mm + 2, :],
                            in0=et.rearrange("p (m q) -> p m q", m=2),
                            in1=maskT_sb[:, 2 * mm : 2 * mm + 2, :],
                            op=mybir.AluOpType.mult,
                        )
                        if pend is not None:
                            emit_pv_chunk(pend, mm)
                        # spread the previous qq's projection one j-block per
                        # head (at mm==5, after that j's PV is finalized) so
                        # per-head PE work stays balanced against the exps
                        if pending_proj is not None and mm == 5:
                            pq, pattn, nj = pending_proj
                            emit_proj_j(pq, pattn, nj)
                            pending_proj = None if nj == 3 else (pq, pattn, nj + 1)
                    pend = {
                        "probsT": probsT,
                        "h": h,
                        "attn": attn_tiles,
                        "qq": qq,
                        "pv": {},
                    }
                    if h == H - 1 and qq < 3:
                        pending_proj = (qq, attn_tiles, 0)
            # tail: flush PV for (qq=3, h=3) and its projection
            for mm in range(8):
                emit_pv_chunk(pend, mm)
            for j in range(4):
                emit_proj_j(3, pend["attn"], j)

    nc.compile()
    return nc


def _get_nc():
    if "nc" not in _CACHE:
        _CACHE["nc"] = _build_nc()
    return _CACHE["nc"]


def _prep_host(x, pos_emb, ln_w, ln_b, aff_w, aff_b, W_qkv, mask_table, W_proj):
    f = np.float32
    bf = ml_dtypes.bfloat16
    x = np.asarray(x, f)
    pos_emb = np.asarray(pos_emb)
    ln_w = np.asarray(ln_w, f)
    ln_b = np.asarray(ln_b, f)
    aff_w = np.asarray(aff_w, f)
    aff_b = np.asarray(aff_b, f)
    W_qkv = np.asarray(W_qkv, f)
    mask_table = np.asarray(mask_table)
    W_proj = np.asarray(W_proj, f)

    s = ln_w * aff_w
    c = ln_b * aff_w + aff_b
    Wf = (s[:, None] * W_qkv).astype(f)
    bf_ = (c @ W_qkv).astype(f)
    scale = f(1.0 / np.sqrt(HD))
    Wqk = np.concatenate([Wf[:, :N] * scale, Wf[:, N : 2 * N]], axis=1)
    Wqk = np.ascontiguousarray(Wqk.reshape(KC, P, 2 * N)).astype(bf)
    bqk = np.concatenate([bf_[:N] * scale, bf_[N : 2 * N]]).astype(f)
    Wv = np.ascontiguousarray(Wf[:, 2 * N :]).astype(bf)
    bv = bf_[2 * N :]
    obias = np.ascontiguousarray(
        np.broadcast_to((bv @ W_proj).astype(f), (P, N))
    )
    Wp = np.ascontiguousarray(W_proj).astype(bf)
    # transposed multiplicative mask, pre-tiled:
    # maskt[qq, p, m, q'] = mask01[qq*512+q', m*128+p]
    maskmt = np.where(mask_table[pos_emb], f(1.0), f(0.0)).T  # [k, q]
    maskt = np.ascontiguousarray(
        maskmt.reshape(NT, P, 4, 512).transpose(2, 1, 0, 3)
    ).astype(bf)
    return x, maskt, Wqk, Wv, Wp, bqk, obias


def _install_ntff_hook():
    """Provide the antenv.axon_hooks shim missing from this image so
    run_bass_kernel_spmd(trace=True) can capture NTFF profiles."""
    import types

    try:
        from antenv.axon_hooks import get_axon_ntff_profile_hook  # noqa: F401

        return
    except ImportError:
        pass
    try:
        import antenv
        from trn_agent_boot.trn_boot import _ntff_profile_via_ctypes

        hook = _ntff_profile_via_ctypes("/opt/axon/libaxon_pjrt.so")
        mod = types.ModuleType("antenv.axon_hooks")
        _h = [hook]
        mod.set_axon_ntff_profile_hook = lambda h: _h.__setitem__(0, h)
        mod.get_axon_ntff_profile_hook = lambda: _h[0]
        sys.modules["antenv.axon_hooks"] = mod
        antenv.axon_hooks = mod
    except Exception as e:  # pragma: no cover
        print(f"ntff hook install failed: {e}")


def kernel(x, pos_emb, ln_w, ln_b, aff_w, aff_b, W_qkv, mask_table, W_proj):
    global LAST_RESULTS
    from concourse.bass_utils import run_bass_kernel_spmd

    x, maskt, Wqk, Wv, Wp, bqk, obias = _prep_host(
        x, pos_emb, ln_w, ln_b, aff_w, aff_b, W_qkv, mask_table, W_proj
    )
    nc = _get_nc()
    in_maps = [
        {
            "x": np.ascontiguousarray(x[i]),
            "maskt": maskt,
            "wqk": Wqk,
            "wv": Wv,
            "wproj": Wp,
            "bqk": bqk,
            "obias": obias,
        }
        for i in range(B)
    ]
    trace = bool(int(os.environ.get("ATTN_TRACE", "0")))
    if trace:
        _install_ntff_hook()
    res = run_bass_kernel_spmd(
        nc, in_maps, core_ids=list(range(B)), trace=trace
    )
    LAST_RESULTS = res
    out = np.stack([np.asarray(r["out"]) for r in res.results], axis=0)
    return out.astype(np.float32)
